# revision 1
# baseline (speedup 1.0000x reference)
"""Trainium2 Bass kernel for nn_DynamicConv (dense_cnn).

out[i, j, co, h, w] = sum_k (conv_k(x_i)[co, h, w] + b_k[co]) * attn[j, k]
attn = softmax(softmax(MLP(meanpool(x)), k) / TAU, k)

Sharding: data-parallel over batch i across 8 cores.  Each core convolves its
own sample (9 shifted matmuls over a zero-padded image, contraction = CIN=128,
fp32r) and computes the full [B, K] attention matrix locally from a replicated
copy of x (it is tiny), then applies the cross-batch blend as one
block-diagonal matmul per 16-channel group:
  contraction 64 = (k=4) x (co16), M = 128 = (j=8) x (co16).
Conv weights are host-packed so output channels land in (co, k)-interleaved
partition order, which makes the blend's rhs a contiguous partition range.
All matmul operands are float32r (FP22 multiply, fp32 accumulate) — full PE
rate; the BIR verifier requires producers of those tiles to emit float32r.
"""

import sys

import numpy as np

if "/opt/trn_rl_repo" not in sys.path:
    sys.path.insert(0, "/opt/trn_rl_repo")

import concourse.bacc as bacc
import concourse.bass as bass
import concourse.mybir as mybir
import concourse.tile as tile

F32 = mybir.dt.float32
F32R = mybir.dt.float32r
AF = mybir.ActivationFunctionType
AX = mybir.AxisListType
ALU = mybir.AluOpType

B = 8
CIN = 128
COUT = 256
K = 4
KS = 3
HW = 48
HW2 = HW * HW          # 2304
WP = HW + 2            # 50 (padded)
HID = 256
TAU = 30.0
NCORES = 8

ROW_GROUPS = [(0, 10), (10, 10), (20, 10), (30, 10), (40, 8)]
CHUNKS = [(0, 512), (512, 512), (1024, 512), (1536, 512), (2048, 256)]


def build_nc():
    nc = bacc.Bacc("TRN2", debug=False, num_devices=NCORES)

    xi = nc.dram_tensor("xi", [CIN, HW2], F32R, kind="ExternalInput").ap()
    # [ci, t, tap, p] flattened; p = c*4 + k encodes (co = 32 t + c, k)
    wconv = nc.dram_tensor(
        "wconv", [CIN, 8 * 9 * 128], F32R, kind="ExternalInput"
    ).ap()
    bconv = nc.dram_tensor("bconv", [128, 8], F32, kind="ExternalInput").ap()
    w1t = nc.dram_tensor("w1t", [CIN, HID], F32R, kind="ExternalInput").ap()
    b1c = nc.dram_tensor("b1c", [128, 2], F32, kind="ExternalInput").ap()
    w2t = nc.dram_tensor("w2t", [128, 2 * K], F32R, kind="ExternalInput").ap()
    b2r = nc.dram_tensor("b2r", [1, K], F32R, kind="ExternalInput").ap()
    ident8 = nc.dram_tensor("ident8", [B, B], F32R, kind="ExternalInput").ap()
    # memset can't write float32r tiles (walrus ISA check) — ship constants
    zer128 = nc.dram_tensor("zer128", [128, 128], F32R, kind="ExternalInput").ap()
    one18 = nc.dram_tensor("one18", [1, B], F32R, kind="ExternalInput").ap()
    out = nc.dram_tensor("out", [B, COUT, HW2], F32, kind="ExternalOutput").ap()
    # internal DRAM for the cross-core attention-row AllGather
    cc_in = nc.dram_tensor("cc_in", [1, K], F32).ap()
    cc_out = nc.dram_tensor("cc_out", [B, K], F32, addr_space="Shared").ap()

    with tile.TileContext(nc, num_cores=NCORES) as tc:
        with (
            tc.tile_pool(name="const", bufs=1) as const,
            tc.tile_pool(name="csb", bufs=8) as csb_pool,
            tc.tile_pool(name="osb", bufs=5) as osb_pool,
            tc.tile_pool(name="psA", bufs=3, space="PSUM") as psA,
            tc.tile_pool(name="psB", bufs=4, space="PSUM") as psB,
            tc.tile_pool(name="psM", bufs=1, space="PSUM") as psM,
        ):
            # ---- conv-critical loads first: image, then weights ----
            # each queue moves ~42GB/s (one descriptor per partition line), so
            # split large transfers across queues
            xfull = const.tile([128, HW2], F32R)
            nc.gpsimd.dma_start(xfull[:], xi[:, :])
            ztile = const.tile([128, 128], F32R)
            nc.sync.dma_start(ztile[:], zer128[:, :])

            # pre-warm the ACT function tables (1.3us each if loaded lazily
            # inside the latency-critical chains)
            actw = const.tile([128, 1], F32)
            zcol = ztile[:, 0:1].bitcast(F32)
            nc.scalar.activation(actw[:], zcol, AF.Identity, bias=zcol)
            nc.scalar.activation(actw[:], zcol, AF.Relu, bias=zcol)
            nc.scalar.activation(actw[:], zcol, AF.Exp, bias=zcol)
            nc.scalar.copy(actw[:], zcol)

            # padded image built on-chip (a strided DMA here would shatter
            # into 192B descriptors and swamp the queues)
            xp = const.tile([128, WP * WP], F32R)
            xp3 = xp[:].rearrange("p (h w) -> p h w", w=WP)
            xf3 = xfull[:].rearrange("p (h w) -> p h w", w=HW)
            nc.vector.tensor_copy(xp3[:, 1 : 1 + HW, 1 : 1 + HW], xf3[:, :, :])
            nc.vector.tensor_copy(xp3[:, 0, 0:WP], ztile[:, 0:WP])
            nc.vector.tensor_copy(xp3[:, WP - 1, 0:WP], ztile[:, 0:WP])
            nc.vector.tensor_copy(xp3[:, 1 : 1 + HW, 0], ztile[:, 0:HW])
            nc.vector.tensor_copy(xp3[:, 1 : 1 + HW, WP - 1], ztile[:, 0:HW])

            wt = []
            for t in range(8):
                w = const.tile([128, 9 * 128], F32R, tag=f"wt{t}")
                nc.gpsimd.dma_start(w[:], wconv[:, t * 9 * 128 : (t + 1) * 9 * 128])
                wt.append(w)
            bct = const.tile([128, 8], F32)
            nc.gpsimd.dma_start(bct[:], bconv[:, :])
            w1s = const.tile([128, HID], F32R)
            nc.gpsimd.dma_start(w1s[:], w1t[:, :])
            b1s = const.tile([128, 2], F32)
            nc.gpsimd.dma_start(b1s[:], b1c[:, :])
            w2s = const.tile([128, 2 * K], F32R)
            nc.gpsimd.dma_start(w2s[:], w2t[:, :])
            b2s = const.tile([1, K], F32R)
            nc.gpsimd.dma_start(b2s[:], b2r[:, :])
            id8 = const.tile([B, B], F32R)
            nc.gpsimd.dma_start(id8[:], ident8[:, :])
            ones = const.tile([1, B], F32R)
            nc.gpsimd.dma_start(ones[:], one18[:, :])

            # ---- local global-average pooling (own sample only) ----
            pooled_loc = const.tile([128, 1], F32R)  # [ci, 1] sums; 1/HW2 in w1t
            with nc.allow_low_precision(reason="fp32r matmul operand"):
                nc.vector.tensor_reduce(
                    pooled_loc[:], xfull[:], axis=AX.X, op=ALU.add
                )

            cs_tiles = [None] * 8

            def emit_conv(t):
                cs = csb_pool.tile([128, HW2], F32R, tag="csb")
                cs_tiles[t] = cs
                for (r0, R) in ROW_GROUPS:
                    pt = psA.tile([128, R * HW], F32, tag="cps")
                    for tap in range(9):
                        dh, dw = divmod(tap, 3)
                        rhs = xp3[:, r0 + dh : r0 + dh + R, dw : dw + HW]
                        nc.tensor.matmul(
                            pt[:],
                            lhsT=wt[t][:, tap * 128 : (tap + 1) * 128],
                            rhs=rhs,
                            start=(tap == 0),
                            stop=(tap == 8),
                        )
                    # PSUM -> SBUF eviction, fused with the conv bias add
                    nc.scalar.activation(
                        cs[:, r0 * HW : (r0 + R) * HW],
                        pt[:],
                        AF.Identity,
                        bias=bct[:, t : t + 1],
                    )

            def emit_blend(t, BD):
                cs = cs_tiles[t]
                for u in range(2):
                    g = 2 * t + u
                    ob = osb_pool.tile([128, HW2], F32, tag="osb")
                    for ci_, (c0, C) in enumerate(CHUNKS):
                        bp = psB.tile([128, C], F32, tag="bps")
                        nc.tensor.matmul(
                            bp[:],
                            lhsT=BD[:, 128 * u : 128 * u + 128],
                            rhs=cs[:, c0 : c0 + C],
                            start=True,
                            stop=True,
                        )
                        # PSUM drain balanced across DVE and ACT so psB bank
                        # recycling (not one engine) sets the blend rate
                        if ci_ in (1, 4):
                            nc.scalar.copy(ob[:, c0 : c0 + C], bp[:])
                        else:
                            nc.vector.tensor_copy(ob[:, c0 : c0 + C], bp[:])
                    nc.gpsimd.dma_start(out[:, 16 * g : 16 * g + 16, :], ob[:])

            # ---- attention MLP + double softmax (local row, computed with
            # the same batched shapes that are known to compile: pooled is
            # broadcast to 8 columns, giving 8 identical rows) ----
            pooled8 = const.tile([128, B], F32R)
            nc.vector.tensor_copy(
                pooled8[:], pooled_loc[:, 0:1].broadcast_to([128, B])
            )
            hd = []
            for h in range(2):
                hps = psM.tile([128, B], F32, tag="mlp")
                nc.tensor.matmul(
                    hps[:],
                    lhsT=w1s[:, h * 128 : (h + 1) * 128],
                    rhs=pooled8[:],
                    start=True,
                    stop=True,
                )
                hsb = const.tile([128, B], F32R, tag=f"hd{h}")
                nc.scalar.activation(hsb[:], hps[:], AF.Relu, bias=b1s[:, h : h + 1])
                hd.append(hsb)

            lps = psM.tile([B, K], F32, tag="mlp")
            nc.tensor.matmul(
                lps[:], lhsT=hd[0][:], rhs=w2s[:, 0:K], start=True, stop=False
            )
            nc.tensor.matmul(
                lps[:], lhsT=hd[1][:], rhs=w2s[:, K : 2 * K], start=False, stop=False
            )
            nc.tensor.matmul(
                lps[:], lhsT=ones[:], rhs=b2s[:], start=False, stop=True
            )

            # double softmax over k (shift-invariant: max-subtraction dropped)
            e1 = const.tile([B, K], F32)
            nc.scalar.activation(e1[:], lps[:], AF.Exp, bias=0.0, scale=1.0)
            s1 = const.tile([B, 1], F32)
            nc.vector.tensor_reduce(s1[:], e1[:], axis=AX.X, op=ALU.add)
            r1 = const.tile([B, 1], F32)
            nc.vector.reciprocal(r1[:], s1[:])
            a1 = const.tile([B, K], F32)
            nc.vector.tensor_scalar_mul(a1[:], e1[:], r1[:, 0:1])

            e2 = const.tile([B, K], F32)
            nc.scalar.activation(e2[:], a1[:], AF.Exp, bias=0.0, scale=1.0 / TAU)
            s2 = const.tile([B, 1], F32)
            nc.vector.tensor_reduce(s2[:], e2[:], axis=AX.X, op=ALU.add)
            r2 = const.tile([B, 1], F32)
            nc.vector.reciprocal(r2[:], s2[:])
            attn_loc = const.tile([B, K], F32R)
            nc.vector.tensor_scalar_mul(attn_loc[:], e2[:], r2[:, 0:1])

            # AllGather row 0 of the (identical-row) local attn -> true [B, K]
            nc.sync.dma_start(cc_in.bitcast(F32R), attn_loc[0:1, :])
            nc.gpsimd.collective_compute(
                "AllGather",
                ALU.bypass,
                replica_groups=[list(range(NCORES))],
                ins=[cc_in],
                outs=[cc_out],
            )
            attn = const.tile([B, K], F32R)
            nc.sync.dma_start(attn[:], cc_out.bitcast(F32R))

            # conv t=0..6 on the PE while the AllGather completes (the
            # collective mesh takes ~70-90us wall; hide it under conv work)
            for _t in range(7):
                emit_conv(_t)

            # attn [j, k] -> attn_T [k, j] via PE transpose
            tps = psM.tile([K, B], F32R, tag="mlp")
            nc.tensor.transpose(tps[:], attn[:], id8[:])
            atT = const.tile([K, B], F32R)
            nc.scalar.copy(atT[:], tps[:])

            # full-contraction blend weights: BD2[:, u*128 + j*16 + c] picks
            # attn[j, k] at partition 64u + c*4 + k, zero elsewhere, so the
            # matmul contracts all 128 partitions of cs (zeros are harmless)
            # and lhsT always sits at base_partition 0
            BD2 = const.tile([128, 256], F32R)
            nc.vector.tensor_copy(BD2[:, 0:128], ztile[:])
            nc.vector.tensor_copy(BD2[:, 128:256], ztile[:])
            BDv = BD2[:].rearrange("p (u j c) -> p u j c", c=16, u=2)
            for u in range(2):
                for c in range(16):
                    # compute engines can't start at partition 4c; DMA can
                    p0 = 64 * u + c * 4
                    nc.sync.dma_start(BDv[p0 : p0 + 4, u, :, c], atT[:])
            BD = BD2

            # last conv, then drain all blends
            emit_blend(0, BD)
            emit_conv(7)
            for t in range(1, 8):
                emit_blend(t, BD)

    nc.compile()
    return nc


def pack_inputs(x, conv_w, conv_b, w1, b1, w2, b2):
    """Host-side layout packing (no arithmetic beyond constant folding of the
    mean-pool scale into w1)."""
    x = np.ascontiguousarray(x, dtype=np.float32)
    x_all = x.reshape(B, CIN, HW2)

    # conv_w [K, COUT, CIN, 3, 3] -> [ci, t, tap, p] with p = c*4 + k,
    # co = 32 t + c
    w = np.asarray(conv_w, dtype=np.float32).transpose(2, 3, 4, 0, 1)  # ci kh kw k co
    w = w.reshape(CIN, KS, KS, K, 8, 32)  # ci kh kw k t c
    w = w.transpose(0, 4, 1, 2, 5, 3)  # ci t kh kw c k
    wconv = np.ascontiguousarray(w.reshape(CIN, 8 * 9 * 128))

    bc = np.asarray(conv_b, dtype=np.float32).reshape(K, 8, 32)  # k t c
    bconv = np.ascontiguousarray(bc.transpose(1, 2, 0).reshape(8, 128).T)  # [p, t]

    w1t = np.ascontiguousarray(np.asarray(w1, dtype=np.float32).T) / float(HW2)
    b1c = np.ascontiguousarray(np.asarray(b1, dtype=np.float32).reshape(2, 128).T)
    w2T = np.asarray(w2, dtype=np.float32).T  # [256, 4]
    w2t = np.ascontiguousarray(np.concatenate([w2T[:128], w2T[128:]], axis=1))
    b2r = np.asarray(b2, dtype=np.float32).reshape(1, K)
    ident8 = np.eye(B, dtype=np.float32)

    common = dict(
        wconv=wconv, bconv=bconv, w1t=w1t, b1c=b1c,
        w2t=w2t, b2r=b2r, ident8=ident8,
        zer128=np.zeros((128, 128), dtype=np.float32),
        one18=np.ones((1, B), dtype=np.float32),
    )
    in_maps = [dict(common, xi=np.ascontiguousarray(x_all[i])) for i in range(NCORES)]
    return in_maps


def run(inputs, trace=False):
    from concourse.bass_utils import run_bass_kernel_spmd

    nc = build_nc()
    in_maps = pack_inputs(**inputs)
    res = run_bass_kernel_spmd(
        nc, in_maps, core_ids=list(range(NCORES)), trace=trace
    )
    slabs = [res.results[i]["out"] for i in range(NCORES)]
    out = np.stack(slabs, axis=0).reshape(B, B, COUT, HW, HW)
    return out, res


def kernel(**inputs) -> np.ndarray:
    out, _ = run(inputs, trace=False)
    return out



# revision 2
# speedup vs baseline: 1.4141x; 1.4141x over previous
"""Trainium2 Bass kernel for nn_DynamicConv (dense_cnn).

out[i, j, co, h, w] = sum_k (conv_k(x_i)[co, h, w] + b_k[co]) * attn[j, k]
attn = softmax(softmax(MLP(meanpool(x)), k) / TAU, k)

Sharding: data-parallel over batch i across 8 cores, with NO cross-core
collective.  The attention matrix needs pooled vectors of ALL samples, so
every core receives the full batch in bf16 (4.6 MB) and computes the whole
[B, K] attention locally.  A runtime AllGather was measured to cost
15-105us per core purely in launch-skew rendezvous; replicating the input
removes that entirely and makes the cores embarrassingly parallel.

Per-core xall is ROTATED so slot 0 is the core's own sample (the conv
input); the host un-rotates the output slabs (np.roll) when gathering.

Everything on the PE runs in bf16 (fp32 PSUM accumulate): conv as 9
shifted matmuls over a zero-padded image, then the cross-batch blend as a
block-diagonal matmul per 16-channel group.  The block-diagonal blend
matrix BD is built on-chip as (P4.T @ broadcast(attn.T)) * M01 with two
tiny constants, avoiding 32 scatter DMAs.  Output stores rotate across
the three DMA queues (sync / scalar / gpsimd).
"""

import sys

import numpy as np

if "/opt/trn_rl_repo" not in sys.path:
    sys.path.insert(0, "/opt/trn_rl_repo")

import concourse.bacc as bacc
import concourse.bass as bass
import concourse.mybir as mybir
import concourse.tile as tile

F32 = mybir.dt.float32
BF = mybir.dt.bfloat16
AF = mybir.ActivationFunctionType
AX = mybir.AxisListType
ALU = mybir.AluOpType

B = 8
CIN = 128
COUT = 256
K = 4
KS = 3
HW = 48
HW2 = HW * HW          # 2304
WP = HW + 2            # 50 (padded)
HID = 256
TAU = 30.0
NCORES = 8

ROW_GROUPS = [(0, 10), (10, 10), (20, 10), (30, 10), (40, 8)]
CHUNKS = [(0, 512), (512, 512), (1024, 512), (1536, 512), (2048, 256)]


def build_nc():
    nc = bacc.Bacc("TRN2", debug=False, num_devices=NCORES)

    # slot q holds sample (core + q) % 8; slot 0 is the core's own sample
    xall = nc.dram_tensor("xall", [B * CIN, HW2], BF, kind="ExternalInput").ap()
    # [ci, t, tap, p] flattened; p = c*4 + k encodes (co = 32 t + c, k)
    wconv = nc.dram_tensor("wconv", [CIN, 8 * 9 * 128], BF, kind="ExternalInput").ap()
    bconv = nc.dram_tensor("bconv", [128, 8], F32, kind="ExternalInput").ap()
    w1t = nc.dram_tensor("w1t", [CIN, HID], BF, kind="ExternalInput").ap()
    b1c = nc.dram_tensor("b1c", [128, 2], F32, kind="ExternalInput").ap()
    w2t = nc.dram_tensor("w2t", [128, 2 * K], BF, kind="ExternalInput").ap()
    b2r = nc.dram_tensor("b2r", [1, K], BF, kind="ExternalInput").ap()
    one18 = nc.dram_tensor("one18", [1, B], BF, kind="ExternalInput").ap()
    ident8 = nc.dram_tensor("ident8", [B, B], BF, kind="ExternalInput").ap()
    # p4[k, 64u + 4c + k] = 1: scatters attn.T rows onto the (c, k) comb
    p4d = nc.dram_tensor("p4", [K, 128], BF, kind="ExternalInput").ap()
    # m01[64u + 4c + k, 128u' + 16j + c'] = (u == u') & (c == c')
    m01d = nc.dram_tensor("m01", [128, 256], BF, kind="ExternalInput").ap()
    zerd = nc.dram_tensor("zer", [128, 64], BF, kind="ExternalInput").ap()
    out = nc.dram_tensor("out", [B, COUT, HW2], F32, kind="ExternalOutput").ap()

    with tile.TileContext(nc, num_cores=NCORES) as tc:
        with (
            tc.tile_pool(name="const", bufs=1) as const,
            tc.tile_pool(name="csb", bufs=8) as csb_pool,
            tc.tile_pool(name="osb", bufs=5) as osb_pool,
            tc.tile_pool(name="psA", bufs=3, space="PSUM") as psA,
            tc.tile_pool(name="psB", bufs=3, space="PSUM") as psB,
            tc.tile_pool(name="psM", bufs=1, space="PSUM") as psM,
        ):
            # ---- loads: own image + first weights first ----
            xall_sb = const.tile([128, B * HW2], BF)
            nc.sync.dma_start(xall_sb[:, 0:HW2], xall[0:128, :])
            zer = const.tile([128, 64], BF)
            nc.scalar.dma_start(zer[:], zerd[:, :])
            wt = []
            for t in range(8):
                w = const.tile([128, 9 * 128], BF, tag=f"wt{t}")
                nc.gpsimd.dma_start(w[:], wconv[:, t * 9 * 128 : (t + 1) * 9 * 128])
                wt.append(w)
            for j in range(1, 8):
                eng = nc.scalar if j % 2 else nc.sync
                eng.dma_start(
                    xall_sb[:, j * HW2 : (j + 1) * HW2],
                    xall[j * 128 : (j + 1) * 128, :],
                )
            bct = const.tile([128, 8], F32)
            nc.gpsimd.dma_start(bct[:], bconv[:, :])
            w1s = const.tile([128, HID], BF)
            nc.scalar.dma_start(w1s[:], w1t[:, :])
            b1s = const.tile([128, 2], F32)
            nc.scalar.dma_start(b1s[:], b1c[:, :])
            w2s = const.tile([128, 2 * K], BF)
            nc.scalar.dma_start(w2s[:], w2t[:, :])
            b2s = const.tile([1, K], BF)
            nc.scalar.dma_start(b2s[:], b2r[:, :])
            ones = const.tile([1, B], BF)
            nc.scalar.dma_start(ones[:], one18[:, :])
            id8 = const.tile([B, B], BF)
            nc.scalar.dma_start(id8[:], ident8[:, :])
            p4s = const.tile([K, 128], BF)
            nc.scalar.dma_start(p4s[:], p4d[:, :])
            m01 = const.tile([128, 256], BF)
            nc.scalar.dma_start(m01[:], m01d[:, :])

            # pre-warm the ACT function tables (1.3us each if loaded lazily
            # inside the latency-critical chains)
            actw = const.tile([128, 1], F32)
            zcol = zer[:, 0:2].bitcast(F32)[:, 0:1]
            nc.scalar.activation(actw[:], zcol, AF.Identity, bias=zcol)
            nc.scalar.activation(actw[:], zcol, AF.Relu, bias=zcol)
            nc.scalar.activation(actw[:], zcol, AF.Exp, bias=zcol)
            nc.scalar.copy(actw[:], zcol)

            # padded image built on-chip (a strided DMA here would shatter
            # into tiny descriptors and swamp the queues)
            xp = const.tile([128, WP * WP], BF)
            xp3 = xp[:].rearrange("p (h w) -> p h w", w=WP)
            xf3 = xall_sb[:, 0:HW2].rearrange("p (h w) -> p h w", w=HW)
            nc.vector.tensor_copy(xp3[:, 1 : 1 + HW, 1 : 1 + HW], xf3[:, :, :])
            nc.vector.tensor_copy(xp3[:, 0, 0:WP], zer[:, 0:WP])
            nc.vector.tensor_copy(xp3[:, WP - 1, 0:WP], zer[:, 0:WP])
            nc.vector.tensor_copy(xp3[:, 1 : 1 + HW, 0], zer[:, 0:HW])
            nc.vector.tensor_copy(xp3[:, 1 : 1 + HW, WP - 1], zer[:, 0:HW])

            # ---- global average pooling of all 8 samples (1/HW2 in w1t) ----
            pooled8 = const.tile([128, B], BF)
            with nc.allow_low_precision(reason="bf16 matmul operand"):
                for j in range(8):
                    nc.vector.tensor_reduce(
                        pooled8[:, j : j + 1],
                        xall_sb[:, j * HW2 : (j + 1) * HW2],
                        axis=AX.X,
                        op=ALU.add,
                    )

            cs_tiles = [None] * 8
            store_engs = [nc.sync, nc.scalar, nc.gpsimd]

            def emit_conv(t):
                cs = csb_pool.tile([128, HW2], BF, tag="csb")
                cs_tiles[t] = cs
                for (r0, R) in ROW_GROUPS:
                    pt = psA.tile([128, R * HW], F32, tag="cps")
                    for tap in range(9):
                        dh, dw = divmod(tap, 3)
                        rhs = xp3[:, r0 + dh : r0 + dh + R, dw : dw + HW]
                        nc.tensor.matmul(
                            pt[:],
                            lhsT=wt[t][:, tap * 128 : (tap + 1) * 128],
                            rhs=rhs,
                            start=(tap == 0),
                            stop=(tap == 8),
                        )
                    # PSUM -> SBUF eviction, fused with the conv bias add
                    nc.scalar.activation(
                        cs[:, r0 * HW : (r0 + R) * HW],
                        pt[:],
                        AF.Identity,
                        bias=bct[:, t : t + 1],
                    )

            def emit_blend(t, BD):
                cs = cs_tiles[t]
                for u in range(2):
                    g = 2 * t + u
                    ob = osb_pool.tile([128, HW2], F32, tag="osb")
                    for ci_, (c0, C) in enumerate(CHUNKS):
                        bp = psB.tile([128, C], F32, tag="bps")
                        nc.tensor.matmul(
                            bp[:],
                            lhsT=BD[:, 128 * u : 128 * u + 128],
                            rhs=cs[:, c0 : c0 + C],
                            start=True,
                            stop=True,
                        )
                        # PSUM drain balanced across DVE and ACT so psB bank
                        # recycling (not one engine) sets the blend rate
                        if ci_ in (1, 4):
                            nc.scalar.copy(ob[:, c0 : c0 + C], bp[:])
                        else:
                            nc.vector.tensor_copy(ob[:, c0 : c0 + C], bp[:])
                    store_engs[g % 3].dma_start(out[:, 16 * g : 16 * g + 16, :], ob[:])

            emit_conv(0)
            emit_conv(1)
            emit_conv(2)

            # ---- attention MLP + double softmax for all 8 samples ----
            hd = []
            for h in range(2):
                hps = psM.tile([128, B], F32, tag="mlp")
                nc.tensor.matmul(
                    hps[:],
                    lhsT=w1s[:, h * 128 : (h + 1) * 128],
                    rhs=pooled8[:],
                    start=True,
                    stop=True,
                )
                hsb = const.tile([128, B], BF, tag=f"hd{h}")
                nc.scalar.activation(hsb[:], hps[:], AF.Relu, bias=b1s[:, h : h + 1])
                hd.append(hsb)

            lps = psM.tile([B, K], F32, tag="mlp")
            nc.tensor.matmul(
                lps[:], lhsT=hd[0][:], rhs=w2s[:, 0:K], start=True, stop=False
            )
            nc.tensor.matmul(
                lps[:], lhsT=hd[1][:], rhs=w2s[:, K : 2 * K], start=False, stop=False
            )
            nc.tensor.matmul(
                lps[:], lhsT=ones[:], rhs=b2s[:], start=False, stop=True
            )

            # double softmax over k (shift-invariant: max-subtraction dropped)
            e1 = const.tile([B, K], F32)
            nc.scalar.activation(e1[:], lps[:], AF.Exp, bias=0.0, scale=1.0)
            s1 = const.tile([B, 1], F32)
            nc.vector.tensor_reduce(s1[:], e1[:], axis=AX.X, op=ALU.add)
            r1 = const.tile([B, 1], F32)
            nc.vector.reciprocal(r1[:], s1[:])
            a1 = const.tile([B, K], F32)
            nc.vector.tensor_scalar_mul(a1[:], e1[:], r1[:, 0:1])

            e2 = const.tile([B, K], F32)
            nc.scalar.activation(e2[:], a1[:], AF.Exp, bias=0.0, scale=1.0 / TAU)
            s2 = const.tile([B, 1], F32)
            nc.vector.tensor_reduce(s2[:], e2[:], axis=AX.X, op=ALU.add)
            r2 = const.tile([B, 1], F32)
            nc.vector.reciprocal(r2[:], s2[:])
            attn_bf = const.tile([B, K], BF)
            with nc.allow_low_precision(reason="bf16 blend operand"):
                nc.vector.tensor_scalar_mul(attn_bf[:], e2[:], r2[:, 0:1])

            # attn [j, k] -> attn_T [k, j] via PE transpose
            tps = psM.tile([K, B], BF, tag="mlp")
            nc.tensor.transpose(tps[:], attn_bf[:], id8[:])
            atT = const.tile([K, B], BF)
            nc.scalar.copy(atT[:], tps[:])

            # BD[64u+4c+k, 128u'+16j+c'] = attn[j, k] * (u==u') * (c==c'):
            # comb matmul broadcasts attn.T to every (c, u) slot, the mask
            # kills the off-diagonal (c != c') copies
            atRep = const.tile([K, 256], BF)
            at4 = atT[:].rearrange("k (u j c) -> k u j c", u=1, c=1)
            nc.vector.tensor_copy(
                atRep[:].rearrange("k (u j c) -> k u j c", u=2, c=16),
                at4.broadcast_to([K, 2, 8, 16]),
            )
            psD = psM.tile([128, 256], F32, tag="mlp")
            nc.tensor.matmul(psD[:], lhsT=p4s[:], rhs=atRep[:], start=True, stop=True)
            BD = const.tile([128, 256], BF)
            with nc.allow_low_precision(reason="bf16 blend operand"):
                nc.vector.tensor_tensor(BD[:], psD[:], m01[:], op=ALU.mult)

            # interleave: blend t lags conv by 3 so attn is ready and the
            # PSUM drains of blend t overlap conv t+3's matmuls
            emit_blend(0, BD)
            for t in range(3, 8):
                emit_conv(t)
                emit_blend(t - 2, BD)
            emit_blend(6, BD)
            emit_blend(7, BD)

    nc.compile()
    return nc


def pack_inputs(x, conv_w, conv_b, w1, b1, w2, b2):
    """Host-side layout packing (dtype casts and constant folding only)."""
    import ml_dtypes

    bf16 = ml_dtypes.bfloat16
    x_bf = np.asarray(x, dtype=np.float32).reshape(B, CIN, HW2).astype(bf16)

    # conv_w [K, COUT, CIN, 3, 3] -> [ci, t, tap, p] with p = c*4 + k,
    # co = 32 t + c
    w = np.asarray(conv_w, dtype=np.float32).transpose(2, 3, 4, 0, 1)  # ci kh kw k co
    w = w.reshape(CIN, KS, KS, K, 8, 32)  # ci kh kw k t c
    w = w.transpose(0, 4, 1, 2, 5, 3)  # ci t kh kw c k
    wconv = np.ascontiguousarray(w.reshape(CIN, 8 * 9 * 128)).astype(bf16)

    bc = np.asarray(conv_b, dtype=np.float32).reshape(K, 8, 32)  # k t c
    bconv = np.ascontiguousarray(bc.transpose(1, 2, 0).reshape(8, 128).T)  # [p, t]

    w1t = (np.ascontiguousarray(np.asarray(w1, dtype=np.float32).T) / float(HW2)).astype(bf16)
    b1c = np.ascontiguousarray(np.asarray(b1, dtype=np.float32).reshape(2, 128).T)
    w2T = np.asarray(w2, dtype=np.float32).T  # [256, 4]
    w2t = np.ascontiguousarray(np.concatenate([w2T[:128], w2T[128:]], axis=1)).astype(bf16)
    b2r = np.asarray(b2, dtype=np.float32).reshape(1, K).astype(bf16)

    p4 = np.zeros((K, 128), dtype=np.float32)
    m01 = np.zeros((128, 256), dtype=np.float32)
    for u in range(2):
        for c in range(16):
            for k in range(K):
                p4[k, 64 * u + 4 * c + k] = 1.0
                m01[64 * u + 4 * c + k, 128 * u + 16 * np.arange(8) + c] = 1.0

    common = dict(
        wconv=wconv, bconv=bconv, w1t=w1t, b1c=b1c, w2t=w2t, b2r=b2r,
        one18=np.ones((1, B), dtype=np.float32).astype(bf16),
        ident8=np.eye(B, dtype=np.float32).astype(bf16),
        p4=p4.astype(bf16), m01=m01.astype(bf16),
        zer=np.zeros((128, 64), dtype=np.float32).astype(bf16),
    )
    in_maps = [
        dict(
            common,
            xall=np.ascontiguousarray(
                np.roll(x_bf, -i, axis=0).reshape(B * CIN, HW2)
            ),
        )
        for i in range(NCORES)
    ]
    return in_maps


def run(inputs, trace=False):
    from concourse.bass_utils import run_bass_kernel_spmd

    nc = build_nc()
    in_maps = pack_inputs(**inputs)
    res = run_bass_kernel_spmd(
        nc, in_maps, core_ids=list(range(NCORES)), trace=trace
    )
    # core i's slab row q holds sample j = (i + q) % 8: un-rotate
    slabs = [np.roll(res.results[i]["out"], i, axis=0) for i in range(NCORES)]
    out = np.stack(slabs, axis=0).reshape(B, B, COUT, HW, HW)
    return out, res


def kernel(**inputs) -> np.ndarray:
    out, _ = run(inputs, trace=False)
    return out


# revision 11
# speedup vs baseline: 1.7117x; 1.2105x over previous
"""Trainium2 Bass kernel for nn_DynamicConv (dense_cnn).

out[i, j, co, h, w] = sum_k (conv_k(x_i)[co, h, w] + b_k[co]) * attn[j, k]
attn = softmax(softmax(MLP(meanpool(x)), k) / TAU, k)

Sharding: data-parallel over batch i across 8 cores, with NO cross-core
collective.  The attention matrix needs pooled vectors of ALL samples, so
every core receives the full batch in bf16 (4.6 MB) and computes the whole
[B, K] attention locally.  A runtime AllGather was measured to cost
15-105us per core purely in launch-skew rendezvous; replicating the input
removes that entirely and makes the cores embarrassingly parallel.

Per-core xall is ROTATED so slot 0 is the core's own sample (the conv
input); the host un-rotates the output slabs (np.roll) when gathering.

Everything on the PE runs in bf16 (fp32 PSUM accumulate): conv as 9
shifted matmuls over a zero-padded image, then the cross-batch blend as a
block-diagonal matmul per 16-channel group.  The block-diagonal blend
matrix BD is built on-chip as (P4.T @ broadcast(attn.T)) * M01 with two
tiny constants, avoiding 32 scatter DMAs.  Output stores rotate across
the three DMA queues (sync / scalar / gpsimd).
"""

import sys

import numpy as np

if "/opt/trn_rl_repo" not in sys.path:
    sys.path.insert(0, "/opt/trn_rl_repo")

import concourse.bacc as bacc
import concourse.bass as bass
import concourse.mybir as mybir
import concourse.tile as tile

F32 = mybir.dt.float32
BF = mybir.dt.bfloat16
AF = mybir.ActivationFunctionType
AX = mybir.AxisListType
ALU = mybir.AluOpType

B = 8
CIN = 128
COUT = 256
K = 4
KS = 3
HW = 48
HW2 = HW * HW          # 2304
WP = HW + 2            # 50 (padded)
HID = 256
TAU = 30.0
NCORES = 8

ROW_GROUPS = [(0, 10), (10, 10), (20, 10), (30, 10), (40, 8)]
CHUNKS = [(0, 512), (512, 512), (1024, 512), (1536, 512), (2048, 256)]


def build_nc():
    nc = bacc.Bacc("TRN2", debug=False, num_devices=NCORES)

    # slot q holds sample (core + q) % 8; slot 0 is the core's own sample
    xall = nc.dram_tensor("xall", [B * CIN, HW2], BF, kind="ExternalInput").ap()
    # [ci, t, tap, p] flattened; p = c*4 + k encodes (co = 32 t + c, k)
    wconv = nc.dram_tensor("wconv", [CIN, 8 * 9 * 128], BF, kind="ExternalInput").ap()
    bconv = nc.dram_tensor("bconv", [128, 8], F32, kind="ExternalInput").ap()
    w1t = nc.dram_tensor("w1t", [CIN, HID], BF, kind="ExternalInput").ap()
    b1c = nc.dram_tensor("b1c", [128, 2], F32, kind="ExternalInput").ap()
    w2t = nc.dram_tensor("w2t", [128, 2 * K], BF, kind="ExternalInput").ap()
    b2r = nc.dram_tensor("b2r", [1, K], BF, kind="ExternalInput").ap()
    one18 = nc.dram_tensor("one18", [1, B], BF, kind="ExternalInput").ap()
    ident8 = nc.dram_tensor("ident8", [B, B], BF, kind="ExternalInput").ap()
    # p4[k, 64u + 4c + k] = 1: scatters attn.T rows onto the (c, k) comb
    p4d = nc.dram_tensor("p4", [K, 128], BF, kind="ExternalInput").ap()
    # m01[64u + 4c + k, 128u' + 16j + c'] = (u == u') & (c == c')
    m01d = nc.dram_tensor("m01", [128, 256], BF, kind="ExternalInput").ap()
    zerd = nc.dram_tensor("zer", [128, 64], BF, kind="ExternalInput").ap()
    out = nc.dram_tensor("out", [B, COUT, HW2], F32, kind="ExternalOutput").ap()

    with tile.TileContext(nc, num_cores=NCORES) as tc:
        with (
            tc.tile_pool(name="const", bufs=1) as const,
            tc.tile_pool(name="csb", bufs=8) as csb_pool,
            tc.tile_pool(name="osb", bufs=5) as osb_pool,
            tc.tile_pool(name="psA", bufs=3, space="PSUM") as psA,
            tc.tile_pool(name="psB", bufs=3, space="PSUM") as psB,
            tc.tile_pool(name="psM", bufs=1, space="PSUM") as psM,
        ):
            # ---- loads: ALL on the sync HWDGE queue, ordered by first use.
            # scalar carries zero DMAs (a DMA backlog there was measured to
            # block conv PSUM evictions for 15us); gpsimd is reserved for
            # the 16 output stores (its software queue fans out over all 16
            # DMA engines -- the HWDGE queues only reach 8).
            xall_sb = const.tile([128, B * HW2], BF)
            wt = []
            for t in range(8):
                w = const.tile([128, 9 * 128], BF, tag=f"wt{t}")
                wt.append(w)
            zer = const.tile([128, 64], BF)
            bct = const.tile([128, 8], F32)
            w1s = const.tile([128, HID], BF)
            b1s = const.tile([128, 2], F32)
            w2s = const.tile([128, 2 * K], BF)
            b2s = const.tile([1, K], BF)
            ones = const.tile([1, B], BF)
            id8 = const.tile([B, B], BF)
            p4s = const.tile([K, 128], BF)
            m01 = const.tile([128, 256], BF)

            def ldx(j):
                nc.sync.dma_start(
                    xall_sb[:, j * HW2 : (j + 1) * HW2],
                    xall[j * 128 : (j + 1) * 128, :],
                )

            def ldw(t):
                nc.sync.dma_start(
                    wt[t][:], wconv[:, t * 9 * 128 : (t + 1) * 9 * 128]
                )

            nc.sync.dma_start(zer[:], zerd[:, :])
            ldx(0)
            ldw(0)
            nc.sync.dma_start(bct[:], bconv[:, :])
            ldw(1)
            for j in range(1, 8):
                ldx(j)
            nc.sync.dma_start(w1s[:], w1t[:, :])
            nc.sync.dma_start(w2s[:], w2t[:, :])
            ldw(2)
            nc.sync.dma_start(b1s[:], b1c[:, :])
            nc.sync.dma_start(b2s[:], b2r[:, :])
            nc.sync.dma_start(ones[:], one18[:, :])
            nc.sync.dma_start(id8[:], ident8[:, :])
            nc.sync.dma_start(p4s[:], p4d[:, :])
            nc.sync.dma_start(m01[:], m01d[:, :])
            for t in range(3, 8):
                ldw(t)

            # pre-warm the ACT function tables (1.3us each if loaded lazily
            # inside the latency-critical chains)
            actw = const.tile([128, 1], F32)
            zcol = zer[:, 0:2].bitcast(F32)[:, 0:1]
            nc.scalar.activation(actw[:], zcol, AF.Identity, bias=zcol)
            nc.scalar.activation(actw[:], zcol, AF.Relu, bias=zcol)
            nc.scalar.activation(actw[:], zcol, AF.Exp, bias=zcol)
            nc.scalar.copy(actw[:], zcol)

            # padded image built on-chip (a strided DMA here would shatter
            # into tiny descriptors and swamp the queues)
            xp = const.tile([128, WP * WP], BF)
            xp3 = xp[:].rearrange("p (h w) -> p h w", w=WP)
            xf3 = xall_sb[:, 0:HW2].rearrange("p (h w) -> p h w", w=HW)
            nc.vector.tensor_copy(xp3[:, 1 : 1 + HW, 1 : 1 + HW], xf3[:, :, :])
            nc.vector.tensor_copy(xp3[:, 0, 0:WP], zer[:, 0:WP])
            nc.vector.tensor_copy(xp3[:, WP - 1, 0:WP], zer[:, 0:WP])
            nc.vector.tensor_copy(xp3[:, 1 : 1 + HW, 0], zer[:, 0:HW])
            nc.vector.tensor_copy(xp3[:, 1 : 1 + HW, WP - 1], zer[:, 0:HW])

            # ---- global average pooling of all 8 samples (1/HW2 in w1t).
            # 2.5us per reduce, serial on DVE (~20us total) -- hidden by
            # lagging the first blend two convs behind (attn ready ~34us,
            # first blend matmul at ~40us)
            pooled8 = const.tile([128, B], BF)
            with nc.allow_low_precision(reason="bf16 matmul operand"):
                for j in range(8):
                    nc.vector.tensor_reduce(
                        pooled8[:, j : j + 1],
                        xall_sb[:, j * HW2 : (j + 1) * HW2],
                        axis=AX.X,
                        op=ALU.add,
                    )

            cs_tiles = [None] * 8

            def emit_conv(t):
                cs = csb_pool.tile([128, HW2], BF, tag="csb")
                cs_tiles[t] = cs
                for (r0, R) in ROW_GROUPS:
                    pt = psA.tile([128, R * HW], F32, tag="cps")
                    for tap in range(9):
                        dh, dw = divmod(tap, 3)
                        rhs = xp3[:, r0 + dh : r0 + dh + R, dw : dw + HW]
                        nc.tensor.matmul(
                            pt[:],
                            lhsT=wt[t][:, tap * 128 : (tap + 1) * 128],
                            rhs=rhs,
                            start=(tap == 0),
                            stop=(tap == 8),
                        )
                    # PSUM -> SBUF eviction, fused with the conv bias add
                    nc.scalar.activation(
                        cs[:, r0 * HW : (r0 + R) * HW],
                        pt[:],
                        AF.Identity,
                        bias=bct[:, t : t + 1],
                    )

            def emit_blend(t, BD):
                cs = cs_tiles[t]
                for u in range(2):
                    g = 2 * t + u
                    ob = osb_pool.tile([128, HW2], F32, tag="osb")
                    for ci_, (c0, C) in enumerate(CHUNKS):
                        bp = psB.tile([128, C], F32, tag="bps")
                        nc.tensor.matmul(
                            bp[:],
                            lhsT=BD[:, 128 * u : 128 * u + 128],
                            rhs=cs[:, c0 : c0 + C],
                            start=True,
                            stop=True,
                        )
                        # PSUM drain balanced across DVE and ACT so psB bank
                        # recycling (not one engine) sets the blend rate
                        if ci_ in (1, 4):
                            nc.scalar.copy(ob[:, c0 : c0 + C], bp[:])
                        else:
                            nc.vector.tensor_copy(ob[:, c0 : c0 + C], bp[:])
                    nc.gpsimd.dma_start(out[:, 16 * g : 16 * g + 16, :], ob[:])

            emit_conv(0)
            emit_conv(1)

            # ---- attention MLP + double softmax for all 8 samples ----
            hd = []
            for h in range(2):
                hps = psM.tile([128, B], F32, tag="mlp")
                nc.tensor.matmul(
                    hps[:],
                    lhsT=w1s[:, h * 128 : (h + 1) * 128],
                    rhs=pooled8[:],
                    start=True,
                    stop=True,
                )
                hsb = const.tile([128, B], BF, tag=f"hd{h}")
                nc.scalar.activation(hsb[:], hps[:], AF.Relu, bias=b1s[:, h : h + 1])
                hd.append(hsb)

            lps = psM.tile([B, K], F32, tag="mlp")
            nc.tensor.matmul(
                lps[:], lhsT=hd[0][:], rhs=w2s[:, 0:K], start=True, stop=False
            )
            nc.tensor.matmul(
                lps[:], lhsT=hd[1][:], rhs=w2s[:, K : 2 * K], start=False, stop=False
            )
            nc.tensor.matmul(
                lps[:], lhsT=ones[:], rhs=b2s[:], start=False, stop=True
            )

            # double softmax over k (shift-invariant: max-subtraction dropped)
            e1 = const.tile([B, K], F32)
            nc.scalar.activation(e1[:], lps[:], AF.Exp, bias=0.0, scale=1.0)
            s1 = const.tile([B, 1], F32)
            nc.vector.tensor_reduce(s1[:], e1[:], axis=AX.X, op=ALU.add)
            r1 = const.tile([B, 1], F32)
            nc.vector.reciprocal(r1[:], s1[:])
            a1 = const.tile([B, K], F32)
            nc.vector.tensor_scalar_mul(a1[:], e1[:], r1[:, 0:1])

            e2 = const.tile([B, K], F32)
            nc.scalar.activation(e2[:], a1[:], AF.Exp, bias=0.0, scale=1.0 / TAU)
            s2 = const.tile([B, 1], F32)
            nc.vector.tensor_reduce(s2[:], e2[:], axis=AX.X, op=ALU.add)
            r2 = const.tile([B, 1], F32)
            nc.vector.reciprocal(r2[:], s2[:])
            attn_bf = const.tile([B, K], BF)
            with nc.allow_low_precision(reason="bf16 blend operand"):
                nc.vector.tensor_scalar_mul(attn_bf[:], e2[:], r2[:, 0:1])

            # attn [j, k] -> attn_T [k, j] via PE transpose
            tps = psM.tile([K, B], BF, tag="mlp")
            nc.tensor.transpose(tps[:], attn_bf[:], id8[:])
            atT = const.tile([K, B], BF)
            nc.scalar.copy(atT[:], tps[:])

            # BD[64u+4c+k, 128u'+16j+c'] = attn[j, k] * (u==u') * (c==c'):
            # comb matmul broadcasts attn.T to every (c, u) slot, the mask
            # kills the off-diagonal (c != c') copies
            atRep = const.tile([K, 256], BF)
            at4 = atT[:].rearrange("k (u j c) -> k u j c", u=1, c=1)
            nc.vector.tensor_copy(
                atRep[:].rearrange("k (u j c) -> k u j c", u=2, c=16),
                at4.broadcast_to([K, 2, 8, 16]),
            )
            psD = psM.tile([128, 256], F32, tag="mlp")
            nc.tensor.matmul(psD[:], lhsT=p4s[:], rhs=atRep[:], start=True, stop=True)
            BD = const.tile([128, 256], BF)
            with nc.allow_low_precision(reason="bf16 blend operand"):
                nc.vector.tensor_tensor(BD[:], psD[:], m01[:], op=ALU.mult)

            # interleave: blend t lags conv by 2 so attn (ready ~32us) never
            # stalls the in-order PE queue, and blend t's PSUM drains overlap
            # conv t+3's matmuls
            emit_conv(2)
            for t in range(3, 8):
                emit_blend(t - 3, BD)
                emit_conv(t)
            emit_blend(5, BD)
            emit_blend(6, BD)
            emit_blend(7, BD)

    nc.compile()
    return nc


def pack_inputs(x, conv_w, conv_b, w1, b1, w2, b2):
    """Host-side layout packing (dtype casts and constant folding only)."""
    import ml_dtypes

    bf16 = ml_dtypes.bfloat16
    x_bf = np.asarray(x, dtype=np.float32).reshape(B, CIN, HW2).astype(bf16)

    # conv_w [K, COUT, CIN, 3, 3] -> [ci, t, tap, p] with p = c*4 + k,
    # co = 32 t + c
    w = np.asarray(conv_w, dtype=np.float32).transpose(2, 3, 4, 0, 1)  # ci kh kw k co
    w = w.reshape(CIN, KS, KS, K, 8, 32)  # ci kh kw k t c
    w = w.transpose(0, 4, 1, 2, 5, 3)  # ci t kh kw c k
    wconv = np.ascontiguousarray(w.reshape(CIN, 8 * 9 * 128)).astype(bf16)

    bc = np.asarray(conv_b, dtype=np.float32).reshape(K, 8, 32)  # k t c
    bconv = np.ascontiguousarray(bc.transpose(1, 2, 0).reshape(8, 128).T)  # [p, t]

    w1t = (np.ascontiguousarray(np.asarray(w1, dtype=np.float32).T) / float(HW2)).astype(bf16)
    b1c = np.ascontiguousarray(np.asarray(b1, dtype=np.float32).reshape(2, 128).T)
    w2T = np.asarray(w2, dtype=np.float32).T  # [256, 4]
    w2t = np.ascontiguousarray(np.concatenate([w2T[:128], w2T[128:]], axis=1)).astype(bf16)
    b2r = np.asarray(b2, dtype=np.float32).reshape(1, K).astype(bf16)

    p4 = np.zeros((K, 128), dtype=np.float32)
    m01 = np.zeros((128, 256), dtype=np.float32)
    for u in range(2):
        for c in range(16):
            for k in range(K):
                p4[k, 64 * u + 4 * c + k] = 1.0
                m01[64 * u + 4 * c + k, 128 * u + 16 * np.arange(8) + c] = 1.0

    common = dict(
        wconv=wconv, bconv=bconv, w1t=w1t, b1c=b1c, w2t=w2t, b2r=b2r,
        one18=np.ones((1, B), dtype=np.float32).astype(bf16),
        ident8=np.eye(B, dtype=np.float32).astype(bf16),
        p4=p4.astype(bf16), m01=m01.astype(bf16),
        zer=np.zeros((128, 64), dtype=np.float32).astype(bf16),
    )
    in_maps = [
        dict(
            common,
            xall=np.ascontiguousarray(
                np.roll(x_bf, -i, axis=0).reshape(B * CIN, HW2)
            ),
        )
        for i in range(NCORES)
    ]
    return in_maps


def run(inputs, trace=False):
    from concourse.bass_utils import run_bass_kernel_spmd

    nc = build_nc()
    in_maps = pack_inputs(**inputs)
    res = run_bass_kernel_spmd(
        nc, in_maps, core_ids=list(range(NCORES)), trace=trace
    )
    # core i's slab row q holds sample j = (i + q) % 8: un-rotate
    slabs = [np.roll(res.results[i]["out"], i, axis=0) for i in range(NCORES)]
    out = np.stack(slabs, axis=0).reshape(B, B, COUT, HW, HW)
    return out, res


def kernel(**inputs) -> np.ndarray:
    out, _ = run(inputs, trace=False)
    return out
